# revision 4
# baseline (speedup 1.0000x reference)
"""Causal attention (B=4, S=2048, D=1024) on 8 Trainium2 NeuronCores.

Sharding: data-parallel over batch (4) x 2 cores per batch.  Global q-tiles
(128 rows, 16 per batch) are dealt round-robin: core h=0 of a pair takes even
tiles, h=1 odd tiles.  The program rounds every q-tile's causal key-extent up
to a multiple of 256 -- tile pair (2j, 2j+1) then shares extent 256*(j+1), so
both cores run the *same* instruction stream (SPMD); residual causal masking
comes in as a per-core additive-mask input.

K/V projections are split across the pair: core h projects keys
[1024h, 1024h+1024) from its `xh` input, then the pair exchanges K^T/V via an
AllGather over internal DRAM (replica groups [[0,1],[2,3],[4,5],[6,7]]).
The Q projection runs while the collective is in flight.

All matmuls run in bf16 with fp32 PSUM accumulation:
  xhT/xqT : PE-transposed activations (d on partitions)
  QT[e,q] = wq^T xq^T / sqrt(D), KT[e,k] = wk^T xh^T, V[k,e] = xh wv
  S[q,k]  = QT^T KT (chunks of <=512 cols in PSUM), + additive mask tail
  P       = exp(S) (scores are O(1) -- max-subtraction unnecessary),
            fused row-sum via activation accum_out
  O[q,e]  = (P^T)^T V accumulated over 128-key tiles, scaled by 1/rowsum
"""

import os

os.environ.setdefault("MYCRO_LOCAL_CACHE", "1")

import numpy as np

import concourse.bacc as bacc
import concourse.tile as tile
from concourse import mybir
from concourse.bass_utils import run_bass_kernel_spmd
from concourse.masks import make_identity

B, S, D = 4, 2048, 1024
P = 128
QL = S // 2          # queries per core
HL = S // 2          # keys per core before the gather
NCORES = 8
DT = D // P          # 8 d-tiles (contraction)
ET = D // P          # 8 e-tiles
ST = S // P          # 16 k-tiles after gather
NQT = QL // P        # 8 q-tiles per core
F32 = mybir.dt.float32
BF16 = mybir.dt.bfloat16
NEG = -30000.0       # additive mask value; exp() underflows to exactly 0
CC_GROUPS = [[0, 1], [2, 3], [4, 5], [6, 7]]


def _chunks(extent):
    out, o = [], 0
    while o < extent:
        w = min(512, extent - o)
        out.append((o, w))
        o += w
    return out


def _body(tc, xh, xq, wq, wk, wv, mask, cc_in, cc_out, out):
    nc = tc.nc
    with (
        tc.tile_pool(name="consts", bufs=1) as consts,
        tc.tile_pool(name="qkv", bufs=1) as qkv,
    ):
        ident = consts.tile([P, P], BF16)
        make_identity(nc, ident)
        mask_sb = consts.tile([P, 256], F32)
        nc.sync.dma_start(mask_sb, mask)

        qT = qkv.tile([P, ET, QL], BF16)   # [e_in, e_tile, q]
        kT = qkv.tile([P, ET, S], BF16)    # [e_in, e_tile, k]
        v = qkv.tile([P, ST, D], BF16)     # [k_in, k_tile, e]

        # ------------------------------ projections ------------------------
        with (
            tc.tile_pool(name="wsb", bufs=2) as wpool,
            tc.tile_pool(name="stage", bufs=4) as stpool,
            tc.tile_pool(name="pmm", bufs=4, space="PSUM") as pmm,
            tc.tile_pool(name="ptr", bufs=4, space="PSUM") as ptr,
        ):
            def load_weight(w_ap):
                wsb = wpool.tile([P, DT, D], BF16, tag="w")
                for d in range(DT):
                    stg = stpool.tile([P, D], F32, tag="stage")
                    nc.sync.dma_start(stg, w_ap[d * P:(d + 1) * P, :])
                    nc.vector.tensor_copy(wsb[:, d, :], stg)
                return wsb

            def load_xT_tile(x_ap, s, dst):
                # dst[:, d, s*128:(s+1)*128] = x_ap[s-tile].T (bf16)
                stg = stpool.tile([P, D], F32, tag="stage")
                nc.sync.dma_start(stg, x_ap[s * P:(s + 1) * P, :])
                xb = stpool.tile([P, D], BF16, tag="cast")
                nc.vector.tensor_copy(xb, stg)
                for d in range(DT):
                    pst = ptr.tile([P, P], BF16, tag="tp")
                    nc.tensor.transpose(pst, xb[:, d * P:(d + 1) * P], ident)
                    nc.vector.tensor_copy(dst[:, d, s * P:(s + 1) * P], pst)

            # ---- local K/V half from xh, then pair AllGather
            with tc.tile_pool(name="loc", bufs=1) as locp:
                xhT = locp.tile([P, DT, HL], BF16, tag="xhT")
                ktloc = locp.tile([P, ET, HL], BF16, tag="ktloc")
                vloc = locp.tile([P, ET, D], BF16, tag="vloc")
                wk_sb = load_weight(wk)
                wv_sb = load_weight(wv)
                for c in range(HL // 512):
                    for s in range(4 * c, 4 * c + 4):
                        load_xT_tile(xh, s, xhT)
                    # KT_loc[e, k] = sum_d wk[d, e] xh[k, d]
                    for e in range(ET):
                        ps = pmm.tile([P, 512], F32, tag="mm")
                        for d in range(DT):
                            nc.tensor.matmul(
                                ps, wk_sb[:, d, e * P:(e + 1) * P],
                                xhT[:, d, c * 512:(c + 1) * 512],
                                start=(d == 0), stop=(d == DT - 1))
                        nc.scalar.copy(ktloc[:, e, c * 512:(c + 1) * 512], ps)
                    # V_loc[k, e] = sum_d xh[k, d] wv[d, e]
                    for k in range(4 * c, 4 * c + 4):
                        for ec in range(D // 512):
                            ps = pmm.tile([P, 512], F32, tag="mm")
                            for d in range(DT):
                                nc.tensor.matmul(
                                    ps, xhT[:, d, k * P:(k + 1) * P],
                                    wv_sb[:, d, ec * 512:(ec + 1) * 512],
                                    start=(d == 0), stop=(d == DT - 1))
                            nc.scalar.copy(vloc[:, k, ec * 512:(ec + 1) * 512],
                                           ps)
                # ship halves: cc_in[0] = KT_loc, cc_in[1] = V_loc
                cin = cc_in.rearrange("two (t p) f -> two p t f", p=P)
                nc.sync.dma_start(cin[0], ktloc)
                nc.sync.dma_start(cin[1], vloc)
            nc.gpsimd.collective_compute(
                "AllGather", mybir.AluOpType.bypass,
                replica_groups=CC_GROUPS, ins=[cc_in], outs=[cc_out])

            # ---- Q while the collective flies
            with tc.tile_pool(name="xqp", bufs=1) as xqp:
                xqT = xqp.tile([P, DT, QL], BF16, tag="xqT")
                wq_sb = load_weight(wq)
                for c in range(QL // 512):
                    for s in range(4 * c, 4 * c + 4):
                        load_xT_tile(xq, s, xqT)
                    for e in range(ET):
                        ps = pmm.tile([P, 512], F32, tag="mm")
                        for d in range(DT):
                            nc.tensor.matmul(
                                ps, wq_sb[:, d, e * P:(e + 1) * P],
                                xqT[:, d, c * 512:(c + 1) * 512],
                                start=(d == 0), stop=(d == DT - 1))
                        nc.scalar.mul(qT[:, e, c * 512:(c + 1) * 512], ps,
                                      1.0 / 32.0)

            # ---- repatriate gathered K^T / V into SBUF
            cout = cc_out.rearrange("four (t p) f -> four p t f", p=P)
            kTr = kT.rearrange("p t (half k) -> p t half k", half=2)
            vr = v.rearrange("p (half t) f -> p half t f", half=2)
            for half in range(2):
                nc.sync.dma_start(kTr[:, :, half, :], cout[2 * half])
                nc.sync.dma_start(vr[:, half], cout[2 * half + 1])

        # ------------------------------ attention --------------------------
        with (
            tc.tile_pool(name="attn", bufs=2) as apool,
            tc.tile_pool(name="ptsb", bufs=4) as ptpool,
            tc.tile_pool(name="stats", bufs=2) as spool,
            tc.tile_pool(name="psS", bufs=2, space="PSUM") as psS,
            tc.tile_pool(name="psT", bufs=2, space="PSUM") as psT,
            tc.tile_pool(name="psO", bufs=2, space="PSUM") as psO,
        ):
            for j in range(NQT):
                ext = 256 * (j + 1)
                chunks = _chunks(ext)
                p_sb = apool.tile([P, ext], BF16, tag="p")
                lsum = spool.tile([P, len(chunks)], F32, tag="lsum")
                for ci, (o, w) in enumerate(chunks):
                    ps = psS.tile([P, w], F32, tag="s")
                    for e in range(ET):
                        nc.tensor.matmul(
                            ps, qT[:, e, j * P:(j + 1) * P], kT[:, e, o:o + w],
                            start=(e == 0), stop=(e == ET - 1))
                    if o + w == ext:
                        nc.vector.tensor_add(
                            ps[:, w - 256:w], ps[:, w - 256:w], mask_sb)
                    nc.scalar.activation(
                        p_sb[:, o:o + w], ps, mybir.ActivationFunctionType.Exp,
                        accum_out=lsum[:, ci:ci + 1])
                l_ = spool.tile([P, 1], F32, tag="l")
                nc.vector.reduce_sum(l_, lsum, axis=mybir.AxisListType.X)
                linv = spool.tile([P, 1], F32, tag="linv")
                nc.vector.reciprocal(linv, l_)

                po = psO.tile([P, D], F32, tag="o")
                nk = ext // P
                for k in range(nk):
                    pt_ps = psT.tile([P, P], BF16, tag="pt")
                    nc.tensor.transpose(pt_ps, p_sb[:, k * P:(k + 1) * P], ident)
                    pt = ptpool.tile([P, P], BF16, tag="ptsb")
                    nc.vector.tensor_copy(pt, pt_ps)
                    for c in range(D // 512):
                        nc.tensor.matmul(
                            po[:, c * 512:(c + 1) * 512], pt,
                            v[:, k, c * 512:(c + 1) * 512],
                            start=(k == 0), stop=(k == nk - 1))
                o_sb = apool.tile([P, D], F32, tag="o")
                for c in range(D // 512):
                    nc.vector.tensor_scalar_mul(
                        o_sb[:, c * 512:(c + 1) * 512],
                        po[:, c * 512:(c + 1) * 512], linv)
                nc.sync.dma_start(out[j * P:(j + 1) * P, :], o_sb)


_PROG = None


def _get_prog():
    global _PROG
    if _PROG is None:
        nc = bacc.Bacc("TRN2", target_bir_lowering=False, debug=False,
                       enable_asserts=False, num_devices=NCORES)
        xh = nc.dram_tensor("xh", (HL, D), F32, kind="ExternalInput").ap()
        xq = nc.dram_tensor("xq", (QL, D), F32, kind="ExternalInput").ap()
        wq = nc.dram_tensor("wq", (D, D), F32, kind="ExternalInput").ap()
        wk = nc.dram_tensor("wk", (D, D), F32, kind="ExternalInput").ap()
        wv = nc.dram_tensor("wv", (D, D), F32, kind="ExternalInput").ap()
        mask = nc.dram_tensor("mask", (P, 256), F32, kind="ExternalInput").ap()
        cc_in = nc.dram_tensor("cc_in", (2, HL, D), BF16, kind="Internal").ap()
        cc_out = nc.dram_tensor("cc_out", (4, HL, D), BF16,
                                kind="Internal").ap()
        out = nc.dram_tensor("out", (QL, D), F32, kind="ExternalOutput").ap()
        with tile.TileContext(nc) as tc:
            _body(tc, xh, xq, wq, wk, wv, mask, cc_in, cc_out, out)
        nc.compile()
        _PROG = nc
    return _PROG


def _mask_np(h):
    r = np.arange(P)[:, None]
    c = np.arange(P)[None, :]
    tri = np.where(c <= r, 0.0, NEG).astype(np.float32)
    m = np.zeros((P, 256), np.float32)
    if h == 0:
        m[:, :P] = tri
        m[:, P:] = NEG
    else:
        m[:, P:] = tri
    return m


def _in_map_for_core(inputs, core):
    b, h = core // 2, core % 2
    xb = np.asarray(inputs["x"], np.float32)[b]
    xqb = np.ascontiguousarray(xb.reshape(NQT, 2, P, D)[:, h].reshape(QL, D))
    xhb = np.ascontiguousarray(xb[h * HL:(h + 1) * HL])
    return {
        "xh": xhb,
        "xq": xqb,
        "wq": np.ascontiguousarray(np.asarray(inputs["wq"], np.float32)),
        "wk": np.ascontiguousarray(np.asarray(inputs["wk"], np.float32)),
        "wv": np.ascontiguousarray(np.asarray(inputs["wv"], np.float32)),
        "mask": _mask_np(h),
    }


def _run(inputs, trace=False, tmpdir=None):
    nc = _get_prog()
    in_maps = [_in_map_for_core(inputs, c) for c in range(NCORES)]
    res = run_bass_kernel_spmd(nc, in_maps, core_ids=list(range(NCORES)),
                               trace=trace, tmpdir=tmpdir)
    outf = np.empty((B, S, D), np.float32)
    for core in range(NCORES):
        b, h = core // 2, core % 2
        o = np.asarray(res.results[core]["out"], np.float32)
        outf[b].reshape(NQT, 2, P, D)[:, h] = o.reshape(NQT, P, D)
    return outf, res


def kernel(x, wq, wk, wv):
    outf, _ = _run({"x": x, "wq": wq, "wk": wk, "wv": wv}, trace=False)
    return outf
